# revision 15
# baseline (speedup 1.0000x reference)
"""ChordMixerBlock Trainium2 kernel.

Math (per batch b):
    h   = gelu(data @ w1 + b1)            # exact gelu
    y   = h @ w2 + b2
    out[l, :] = rotate_chord(y)[l, :] + data[l, :]
where rotate_chord rolls track t (channels [16t, 16t+16)) forward by
s_t = 2^(t-1) positions along L (track 0: no shift; track 15: 2^14 == L
-> no shift).

Sharding: 8 cores = (batch b, L-half j); each core computes y for its own
8192-token chunk in transposed layout [256 d, 8192 l] so the contraction
dim D lands on SBUF partitions (host pre-transposes inputs and transposes
the output back).

Roll handling is entirely layout-based -- no cross-core traffic:
  * acc[c, p] = y[c, p] + b2[c] + dataS[c, p], where dataS is the residual
    pre-rolled by +s_t per track on the HOST (pure sharding-layout prep).
    acc[c, p] is then exactly out[global (c0 + p - s_t) mod L, c] -- a
    complete output value, merely stored at a per-track rotated column.
  * Each core dumps acc verbatim; the HOST undoes the per-track column
    rotation while unsharding (np.roll per 16-channel track), so no
    collective and no boundary exchange is needed on device.

Device program per core (bf16 data path, fp32 accumulate in PSUM; the
upper half of the fc2 contraction runs as one fp8e4 DoubleRow matmul per
512-tile, keeping rel err ~1.5e-2 vs the 2e-2 budget):
  * All elementwise work runs on 1024-col (2-PSUM-bank) tiles: gelu on
    the scalar engine and the b2+residual STT on vector amortize their
    ~250ns per-instruction PSUM-access/seq overhead over twice the
    columns (scalar drops ~44us -> ~34us busy, below the PE's ~48us).
  * A dummy 1-col gelu right at program start pulls the ~1.3us
    ACT_TABLE_LOAD off the critical path (it otherwise serializes with
    the first real gelu).
  * w1 is packed ht-major so the first fc1 matmul only needs the first
    256 cols (64 KiB) of the weight stream plus one 512-col slice of dm.
  * Input DMA is spread over all three queues: sync (HWDGE) carries w1
    and dm cols 0:4096 front-loaded in fine slices, scalar (HWDGE)
    carries dm cols 4096:8192 in two big descriptors issued before the
    first gelu, gpsimd (SWDGE, ~2us fixed cost per op -> few big
    descriptors) carries bias/w2/fp8 packs then the rolled residual with
    its first 2048 cols leading.  This keeps the PE fed from ~1.5us
    after queue start with no mid-kernel stream stalls.
  * 16 dependency-free warmup matmuls on a zeroed scratch tile keep the
    PE busy from queue start so the DVFS ramp (0.65/1.2 -> 2.4 GHz after
    ~3us of continuous activity) burns on scratch, not real work.
  * Main loop: groups of l-chunks ([2048, 2048, 2048, 1024, 1024] cols),
    software-pipelined one group back (fc2(g-1) between fc1(g) blocks)
    so the PE never waits on the scalar engine's gelu chain; the two
    trailing 1024-col groups shorten the end-of-kernel gelu+STT tail.
  * gelu+bias on the scalar engine: h rows 0:256 -> bf16, rows 256:512
    -> fp8 planes of [128, 2, 1024] tiles consumed by DoubleRow matmuls.
  * Finished 2048-col output blocks stream out mid-kernel on the sync /
    gpsimd queues (idle once inputs are in); the last two groups finish
    STT+DMA per 1024-tile across all three queues to shorten the tail.
    Host upcasts bf16 -> fp32.
"""

import sys

sys.path.insert(0, "/opt/trn_rl_repo")

import numpy as np
import ml_dtypes

import concourse.bass as bass
import concourse.bacc as bacc
import concourse.tile as tile
import concourse.mybir as mybir
from concourse import bass_utils

B, L, D, H = 4, 16384, 256, 512
N_CORES = 8
LC = L // 2                      # per-core chunk length
NT, TS = 16, 16                  # tracks, track size
SHIFTS = [0] + [2 ** i for i in range(NT - 1)]
SEFF = [s % L for s in SHIFTS]   # track 15 -> 0
TILE = 512                       # max matmul output width
CH = 1024                        # elementwise chunk width (2 PSUM banks)
NCH = LC // CH                   # 8
# groups of 1024-col chunks; small leading groups let the PE start on
# less input data, small trailing groups shorten the end-of-kernel tail
GROUPS = [(0, 1), (1, 2), (2, 4), (4, 6), (6, 7), (7, 8)]

F32 = mybir.dt.float32
BF16 = mybir.dt.bfloat16
F8 = mybir.dt.float8e4


def _build():
    nc = bacc.Bacc(
        "TRN2", target_bir_lowering=False, debug=False,
        num_devices=N_CORES,
    )

    dataM_h = nc.dram_tensor("dataM", [D, LC], BF16, kind="ExternalInput")
    dataS_h = nc.dram_tensor("dataS", [D, LC], BF16, kind="ExternalInput")
    # w1 packed ht-major: cols [ht*256 + dt*128 :+128] = w1 rows
    # [dt*128:+128], cols [ht*128:+128] -- the first 256 cols are all the
    # first fc1 block needs, so it can start on ~64 KiB of weight stream.
    w1pk_h = nc.dram_tensor("w1pk", [128, 1024], BF16, kind="ExternalInput")
    # w2 cols [ht*256 + k*128 :+128] = w2 rows [ht*128:+128] cols [k*128:+128]
    w2pk_h = nc.dram_tensor("w2pk", [128, 512], BF16, kind="ExternalInput")
    # fc2 rows 256:512 as fp8 DoubleRow pack [ki, ko, m] = w2[256+128*ko+ki, m]
    w28_h = nc.dram_tensor("w28", [128, 2, D], F8, kind="ExternalInput")
    # cols 0:4 = b1 (col ht), cols 4:6 = b2 (col k)
    bpk_h = nc.dram_tensor("bpk", [128, 6], F32, kind="ExternalInput")
    outT_h = nc.dram_tensor("outT", [D, LC], BF16, kind="ExternalOutput")

    with tile.TileContext(nc) as tc:
        with (
            tc.tile_pool(name="const", bufs=1) as cpool,
            tc.tile_pool(name="big", bufs=1) as big,
            tc.tile_pool(name="hbf", bufs=8) as hbfp,
            tc.tile_pool(name="h8", bufs=4) as h8p,
            tc.tile_pool(name="ph", bufs=2, space="PSUM") as php,
            tc.tile_pool(name="py", bufs=2, space="PSUM") as pyp,
        ):
            # --- scratch + early gelu table load ---
            wscr = cpool.tile([128, 128], BF16, tag="wscr")
            ascr = cpool.tile([128, 2], F32, tag="ascr")
            nc.gpsimd.memset(ascr[:], 0)
            nc.gpsimd.memset(wscr[:], 0)
            # dummy 1-col gelu: forces ACT_TABLE_LOAD now, in parallel with
            # the input DMA, instead of before the first real gelu
            nc.scalar.activation(
                ascr[:, 1:2], ascr[:, 0:1],
                mybir.ActivationFunctionType.Gelu, bias=0.0,
            )

            w1pk = cpool.tile([128, 1024], BF16, tag="w1pk")
            w2pk = cpool.tile([128, 512], BF16, tag="w2pk")
            w28sb = cpool.tile([128, 2, D], F8, tag="w28")
            bpk = cpool.tile([128, 6], F32, tag="bpk")

            def w1s(dt, ht):
                o = ht * 256 + dt * 128
                return w1pk[:, o:o + 128]

            def w2s(ht, k):
                o = ht * 256 + k * 128
                return w2pk[:, o:o + 128]

            # --- persistent chunk buffers ---
            dm = [big.tile([128, LC], BF16, tag=f"dm{k}", name=f"dm{k}")
                  for k in range(2)]
            ds = [big.tile([128, LC], BF16, tag=f"ds{k}", name=f"ds{k}")
                  for k in range(2)]
            acc = [big.tile([128, LC], BF16, tag=f"acc{k}", name=f"acc{k}")
                   for k in range(2)]

            # --- input DMA ---
            # The fabric delivers little in the first ~8us (descriptor
            # pipeline depth builds slowly) and SDMA round-robin starves
            # small-run queues when another queue moves big runs.  So: the
            # PE-critical stream (w1 + dm cols 0:4096) is split across BOTH
            # HWDGE queues (sync: k=0, scalar: k=1) so two queues build
            # depth in parallel with nothing big competing; every bulk
            # half that is only needed from ~t+30us rides gpsimd (SWDGE)
            # behind the small packs, ordered by deadline.
            # w1 ht0 block + first dm slice lead sync so the first real
            # matmul is gated on ~0.2 MiB; dm cols 0:4096 are
            # deadline-interleaved over both HWDGE queues so at every point
            # in the consumption order each queue carries half the
            # remaining critical bytes
            nc.sync.dma_start(w1pk[:, 0:256], w1pk_h.ap()[:, 0:256])
            nc.sync.dma_start(dm[0][:, 0:512], dataM_h.ap()[0:128, 0:512])
            nc.scalar.dma_start(dm[1][:, 0:512], dataM_h.ap()[128:256, 0:512])
            nc.scalar.dma_start(dm[0][:, 512:1024],
                                dataM_h.ap()[0:128, 512:1024])
            nc.sync.dma_start(w1pk[:, 256:1024], w1pk_h.ap()[:, 256:1024])
            for i, (k, s0, s1) in enumerate((
                    (1, 512, 1024),
                    (0, 1024, 2048), (1, 1024, 2048),
                    (0, 2048, 3072), (1, 2048, 3072),
                    (0, 3072, 4096), (1, 3072, 4096))):
                eng = (nc.sync, nc.scalar)[i % 2]
                eng.dma_start(dm[k][:, s0:s1],
                              dataM_h.ap()[k * 128:(k + 1) * 128, s0:s1])
            nc.gpsimd.dma_start(bpk[:], bpk_h.ap())
            nc.gpsimd.dma_start(w28sb[:, 0:2, :], w28_h.ap())
            nc.gpsimd.dma_start(w2pk[:], w2pk_h.ap())
            for s0, s1 in ((0, 2048), (2048, 4096)):
                for k in range(2):
                    nc.gpsimd.dma_start(
                        ds[k][:, s0:s1],
                        dataS_h.ap()[k * 128:(k + 1) * 128, s0:s1])
            for k in range(2):
                nc.gpsimd.dma_start(
                    dm[k][:, 4096:LC],
                    dataM_h.ap()[k * 128:(k + 1) * 128, 4096:LC])
            for k in range(2):
                nc.gpsimd.dma_start(
                    ds[k][:, 4096:LC],
                    dataS_h.ap()[k * 128:(k + 1) * 128, 4096:LC])

            # --- PE warmup: keep the PE busy from queue start so the DVFS
            # ramp to 2.4 GHz happens on scratch work ---
            for wi in range(24):
                pw = php.tile([128, CH], F32, tag="ph", name=f"warm{wi}")
                nc.tensor.matmul(
                    pw[:, 0:128], wscr[:], wscr[:], start=True, stop=True,
                )

            # --- main loop ---
            hbf = {}
            h8 = {}

            def fc1_block(g, ht):
                c0, c1 = GROUPS[g]
                # stationary w1 tile loaded once per (dt, ht); ph chunks of
                # 1024 cols so gelu runs 2-bank-wide
                ph = {}
                for dt in range(2):
                    for c in range(c0, c1):
                        if dt == 0:
                            ph[c] = php.tile([128, CH], F32, tag="ph",
                                             name=f"ph_{c}_{ht}")
                        for jj in range(2):
                            sl = slice(c * CH + jj * TILE,
                                       c * CH + (jj + 1) * TILE)
                            nc.tensor.matmul(
                                ph[c][:, jj * TILE:(jj + 1) * TILE],
                                w1s(dt, ht), dm[dt][:, sl],
                                start=(dt == 0), stop=(dt == 1),
                            )
                for c in range(c0, c1):
                    if ht < 2:
                        hb = hbfp.tile([128, CH], BF16, tag="hbf",
                                       name=f"hbf_{c}_{ht}")
                        dst = hb[:]
                        hbf[(c, ht)] = hb
                    else:
                        # h rows 256:512 -> fp8 planes for DoubleRow fc2
                        if ht == 2:
                            h8[c] = h8p.tile([128, 2, CH], F8, tag="h8",
                                             name=f"h8_{c}")
                        dst = h8[c][:, ht - 2, :]
                    nc.scalar.activation(
                        dst, ph[c][:],
                        mybir.ActivationFunctionType.Gelu,
                        bias=bpk[:, ht:ht + 1],
                    )

            # tail-out queues: the final output descriptors only reach
            # ~52 GB/s each (no queue depth), so the tail drains per-512
            # across several queues in parallel.  scalar only takes issues
            # for the very last group (its FIFO has no gelus left by then).
            TAIL_ENG = {4: (nc.sync, nc.gpsimd, nc.sync, nc.gpsimd),
                        5: (nc.scalar, nc.sync, nc.gpsimd, nc.scalar)}

            def fc2_piece(g, k, c):
                # one 1024-chunk of fc2 output half k: 6 matmuls + STT
                dsl = slice(k * 128, (k + 1) * 128)
                py_c = pyp.tile([128, CH], F32, tag="py", name=f"py_{c}_{k}")

                def mms(jj):
                    for ht in range(2):
                        nc.tensor.matmul(
                            py_c[:, jj * TILE:(jj + 1) * TILE],
                            w2s(ht, k),
                            hbf[(c, ht)][:, jj * TILE:(jj + 1) * TILE],
                            start=(ht == 0), stop=False,
                        )
                    nc.tensor.matmul(
                        py_c[:, jj * TILE:(jj + 1) * TILE],
                        w28sb[:, :, dsl],
                        h8[c][:, 0:2, jj * TILE:(jj + 1) * TILE],
                        start=False, stop=True,
                        perf_mode=mybir.MatmulPerfMode.DoubleRow,
                    )

                def stt(sl0, sl1):
                    chs = slice(c * CH + sl0, c * CH + sl1)
                    # acc = (y + b2) + rolled residual, bf16 out
                    nc.vector.scalar_tensor_tensor(
                        acc[k][:, chs], py_c[:, sl0:sl1],
                        bpk[:, 4 + k:5 + k], ds[k][:, chs],
                        mybir.AluOpType.add, mybir.AluOpType.add,
                    )
                    return chs

                if g >= 4:
                    # trailing groups: jj-major so each 512-half finishes
                    # (MMs -> STT -> out) independently; PE writes bank B
                    # while vector drains bank A
                    for jj in range(2):
                        mms(jj)
                        chs = stt(jj * TILE, (jj + 1) * TILE)
                        TAIL_ENG[g][2 * k + jj].dma_start(
                            outT_h.ap()[k * 128:(k + 1) * 128, chs],
                            acc[k][:, chs],
                        )
                    return
                for jj in range(2):
                    mms(jj)
                stt(0, CH)

            def out_block(g, eng):
                c0, c1 = GROUPS[g]
                bsl = slice(c0 * CH, c1 * CH)
                for k in range(2):
                    eng.dma_start(
                        outT_h.ap()[k * 128:(k + 1) * 128, bsl],
                        acc[k][:, bsl],
                    )

            NG = len(GROUPS)
            for g in range(NG + 1):
                # fc2 pieces of group g-1, interleaved one per fc1 block of
                # group g so the PE always has matmul work while gelu frees
                # the next ph buffer
                pieces = []
                if g > 0:
                    pc0, pc1 = GROUPS[g - 1]
                    pieces = [(k, c) for k in range(2) for c in range(pc0, pc1)]
                for ht in range(4):
                    if g < NG:
                        fc1_block(g, ht)
                    # big groups have 4 pieces (one per slot); small groups
                    # have 2 (emit on slots 1 and 3)
                    np_ = len(pieces)
                    if np_ == 4:
                        fc2_piece(g - 1, *pieces[ht])
                    elif np_ == 2 and ht in (1, 3):
                        fc2_piece(g - 1, *pieces[ht // 2])
                if g > 0 and g - 1 < 4:
                    # bulk output for the non-tail groups: sync is idle
                    # once its input slices are out (~18us); gpsimd's queue
                    # drains its input bulk by the time g2/g3 finish
                    out_block(g - 1, nc.sync if g - 1 < 2 else nc.gpsimd)

    nc.compile()
    return nc


_NC = None


def _get_nc():
    global _NC
    if _NC is None:
        _NC = _build()
    return _NC


def make_in_maps(data, w1, b1, w2, b2):
    data = np.asarray(data, dtype=np.float32)
    w1f = np.asarray(w1, dtype=np.float32)
    w2f = np.asarray(w2, dtype=np.float32)
    # w1 packed ht-major (see _build)
    w1pk = np.ascontiguousarray(np.concatenate(
        [w1f[dt * 128:(dt + 1) * 128, ht * 128:(ht + 1) * 128]
         for ht in range(4) for dt in range(2)], axis=1,
    )).astype(ml_dtypes.bfloat16)
    w2pk = np.ascontiguousarray(np.concatenate(
        [w2f[0:128, :], w2f[128:256, :]], axis=1,
    )).astype(ml_dtypes.bfloat16)
    # DoubleRow pack: [ki, ko, m] = w2[256 + 128*ko + ki, m]
    w28 = np.ascontiguousarray(
        w2f[H // 2:].reshape(2, 128, D).transpose(1, 0, 2)
    ).astype(ml_dtypes.float8_e4m3)
    # packed biases: cols 0:4 = b1 (col ht = rows ht*128..), cols 4:6 = b2
    bpk = np.ascontiguousarray(np.concatenate(
        [np.asarray(b1, dtype=np.float32).reshape(4, 128).T,
         np.asarray(b2, dtype=np.float32).reshape(2, 128).T], axis=1,
    ))

    in_maps = []
    for bb in range(B):
        # residual pre-rolled by +s_t per track:
        # rolled[l, c] = data[(l - s_t) mod L, c]
        rolled = np.empty((L, D), dtype=np.float32)
        for t in range(NT):
            cs = slice(t * TS, (t + 1) * TS)
            rolled[:, cs] = np.roll(data[bb, :, cs], SEFF[t], axis=0)
        for j in range(2):
            sl = slice(j * LC, (j + 1) * LC)
            dataM = np.ascontiguousarray(
                data[bb, sl, :].T.astype(ml_dtypes.bfloat16)
            )
            dataS = np.ascontiguousarray(
                rolled[sl, :].T.astype(ml_dtypes.bfloat16)
            )
            in_maps.append({
                "dataM": dataM, "dataS": dataS,
                "w1pk": w1pk, "w2pk": w2pk, "w28": w28, "bpk": bpk,
            })
    return in_maps


def kernel(data, w1, b1, w2, b2):
    nc = _get_nc()
    in_maps = make_in_maps(data, w1, b1, w2, b2)
    res = bass_utils.run_bass_kernel_spmd(
        nc, in_maps, core_ids=list(range(N_CORES))
    )
    out = np.empty((B, L, D), dtype=np.float32)
    for bb in range(B):
        # full[c, g] = out[(g - s_t(c)) mod L, c]; undo per-track rotation
        full = np.concatenate(
            [np.asarray(res.results[2 * bb + j]["outT"], dtype=np.float32)
             for j in range(2)], axis=1,
        )
        for t in range(NT):
            seg = full[t * TS:(t + 1) * TS, :]
            out[bb, :, t * TS:(t + 1) * TS] = np.roll(seg, -SEFF[t], axis=1).T
    return out


# revision 16
# speedup vs baseline: 1.0461x; 1.0461x over previous
"""ChordMixerBlock Trainium2 kernel.

Math (per batch b):
    h   = gelu(data @ w1 + b1)            # exact gelu
    y   = h @ w2 + b2
    out[l, :] = rotate_chord(y)[l, :] + data[l, :]
where rotate_chord rolls track t (channels [16t, 16t+16)) forward by
s_t = 2^(t-1) positions along L (track 0: no shift; track 15: 2^14 == L
-> no shift).

Sharding: 8 cores = (batch b, L-half j); each core computes y for its own
8192-token chunk in transposed layout [256 d, 8192 l] so the contraction
dim D lands on SBUF partitions (host pre-transposes inputs and transposes
the output back).

Roll handling is entirely layout-based -- no cross-core traffic:
  * acc[c, p] = y[c, p] + b2[c] + dataS[c, p], where dataS is the residual
    pre-rolled by +s_t per track on the HOST (pure sharding-layout prep).
    acc[c, p] is then exactly out[global (c0 + p - s_t) mod L, c] -- a
    complete output value, merely stored at a per-track rotated column.
  * Each core dumps acc verbatim; the HOST undoes the per-track column
    rotation while unsharding (np.roll per 16-channel track), so no
    collective and no boundary exchange is needed on device.

Device program per core (bf16 data path, fp32 accumulate in PSUM; the
upper half of the fc2 contraction runs as one fp8e4 DoubleRow matmul per
512-tile, keeping rel err ~1.5e-2 vs the 2e-2 budget):
  * All elementwise work runs on 1024-col (2-PSUM-bank) tiles: gelu on
    the scalar engine and the b2+residual STT on vector amortize their
    ~250ns per-instruction PSUM-access/seq overhead over twice the
    columns (scalar drops ~44us -> ~34us busy, below the PE's ~48us).
  * A dummy 1-col gelu right at program start pulls the ~1.3us
    ACT_TABLE_LOAD off the critical path (it otherwise serializes with
    the first real gelu).
  * w1 is packed ht-major so the first fc1 matmul only needs the first
    256 cols (64 KiB) of the weight stream plus one 512-col slice of dm.
  * Input DMA is spread over all three queues: sync (HWDGE) carries w1
    and dm cols 0:4096 front-loaded in fine slices, scalar (HWDGE)
    carries dm cols 4096:8192 in two big descriptors issued before the
    first gelu, gpsimd (SWDGE, ~2us fixed cost per op -> few big
    descriptors) carries bias/w2/fp8 packs then the rolled residual with
    its first 2048 cols leading.  This keeps the PE fed from ~1.5us
    after queue start with no mid-kernel stream stalls.
  * 16 dependency-free warmup matmuls on a zeroed scratch tile keep the
    PE busy from queue start so the DVFS ramp (0.65/1.2 -> 2.4 GHz after
    ~3us of continuous activity) burns on scratch, not real work.
  * Main loop: groups of l-chunks ([2048, 2048, 2048, 1024, 1024] cols),
    software-pipelined one group back (fc2(g-1) between fc1(g) blocks)
    so the PE never waits on the scalar engine's gelu chain; the two
    trailing 1024-col groups shorten the end-of-kernel gelu+STT tail.
  * gelu+bias on the scalar engine: h rows 0:256 -> bf16, rows 256:512
    -> fp8 planes of [128, 2, 1024] tiles consumed by DoubleRow matmuls.
  * Finished 2048-col output blocks stream out mid-kernel on the sync /
    gpsimd queues (idle once inputs are in); the last two groups finish
    STT+DMA per 1024-tile across all three queues to shorten the tail.
    Host upcasts bf16 -> fp32.
"""

import sys

sys.path.insert(0, "/opt/trn_rl_repo")

import numpy as np
import ml_dtypes

import concourse.bass as bass
import concourse.bacc as bacc
import concourse.tile as tile
import concourse.mybir as mybir
from concourse import bass_utils

B, L, D, H = 4, 16384, 256, 512
N_CORES = 8
LC = L // 2                      # per-core chunk length
NT, TS = 16, 16                  # tracks, track size
SHIFTS = [0] + [2 ** i for i in range(NT - 1)]
SEFF = [s % L for s in SHIFTS]   # track 15 -> 0
TILE = 512                       # max matmul output width
CH = 1024                        # elementwise chunk width (2 PSUM banks)
NCH = LC // CH                   # 8
# groups of 1024-col chunks; small leading groups let the PE start on
# less input data, small trailing groups shorten the end-of-kernel tail
GROUPS = [(0, 1), (1, 2), (2, 4), (4, 6), (6, 7), (7, 8)]

F32 = mybir.dt.float32
BF16 = mybir.dt.bfloat16
F8 = mybir.dt.float8e4


def _build():
    nc = bacc.Bacc(
        "TRN2", target_bir_lowering=False, debug=False,
        num_devices=N_CORES,
    )

    dataM_h = nc.dram_tensor("dataM", [D, LC], BF16, kind="ExternalInput")
    dataS_h = nc.dram_tensor("dataS", [D, LC], BF16, kind="ExternalInput")
    # w1 packed ht-major: cols [ht*256 + dt*128 :+128] = w1 rows
    # [dt*128:+128], cols [ht*128:+128] -- the first 256 cols are all the
    # first fc1 block needs, so it can start on ~64 KiB of weight stream.
    w1pk_h = nc.dram_tensor("w1pk", [128, 1024], BF16, kind="ExternalInput")
    # w2 cols [ht*256 + k*128 :+128] = w2 rows [ht*128:+128] cols [k*128:+128]
    w2pk_h = nc.dram_tensor("w2pk", [128, 512], BF16, kind="ExternalInput")
    # fc2 rows 256:512 as fp8 DoubleRow pack [ki, ko, m] = w2[256+128*ko+ki, m]
    w28_h = nc.dram_tensor("w28", [128, 2, D], F8, kind="ExternalInput")
    # cols 0:4 = b1 (col ht), cols 4:6 = b2 (col k)
    bpk_h = nc.dram_tensor("bpk", [128, 6], F32, kind="ExternalInput")
    outT_h = nc.dram_tensor("outT", [D, LC], BF16, kind="ExternalOutput")

    with tile.TileContext(nc) as tc:
        with (
            tc.tile_pool(name="const", bufs=1) as cpool,
            tc.tile_pool(name="big", bufs=1) as big,
            tc.tile_pool(name="hbf", bufs=8) as hbfp,
            tc.tile_pool(name="h8", bufs=4) as h8p,
            tc.tile_pool(name="ph", bufs=2, space="PSUM") as php,
            tc.tile_pool(name="py", bufs=2, space="PSUM") as pyp,
        ):
            # --- scratch + early gelu table load ---
            wscr = cpool.tile([128, 128], BF16, tag="wscr")
            ascr = cpool.tile([128, 2], F32, tag="ascr")
            nc.gpsimd.memset(ascr[:], 0)
            nc.gpsimd.memset(wscr[:], 0)
            # dummy 1-col gelu: forces ACT_TABLE_LOAD now, in parallel with
            # the input DMA, instead of before the first real gelu
            nc.scalar.activation(
                ascr[:, 1:2], ascr[:, 0:1],
                mybir.ActivationFunctionType.Gelu, bias=0.0,
            )

            w1pk = cpool.tile([128, 1024], BF16, tag="w1pk")
            w2pk = cpool.tile([128, 512], BF16, tag="w2pk")
            w28sb = cpool.tile([128, 2, D], F8, tag="w28")
            bpk = cpool.tile([128, 6], F32, tag="bpk")

            def w1s(dt, ht):
                o = ht * 256 + dt * 128
                return w1pk[:, o:o + 128]

            def w2s(ht, k):
                o = ht * 256 + k * 128
                return w2pk[:, o:o + 128]

            # --- persistent chunk buffers ---
            dm = [big.tile([128, LC], BF16, tag=f"dm{k}", name=f"dm{k}")
                  for k in range(2)]
            ds = [big.tile([128, LC], BF16, tag=f"ds{k}", name=f"ds{k}")
                  for k in range(2)]
            acc = [big.tile([128, LC], BF16, tag=f"acc{k}", name=f"acc{k}")
                   for k in range(2)]

            # --- input DMA ---
            # The fabric delivers little in the first ~8us (descriptor
            # pipeline depth builds slowly) and SDMA round-robin starves
            # small-run queues when another queue moves big runs.  So: the
            # PE-critical stream (w1 + dm cols 0:4096) is split across BOTH
            # HWDGE queues (sync: k=0, scalar: k=1) so two queues build
            # depth in parallel with nothing big competing; every bulk
            # half that is only needed from ~t+30us rides gpsimd (SWDGE)
            # behind the small packs, ordered by deadline.
            # Early HWDGE descriptors must be FEW: per-descriptor overhead
            # dominates the first ~10us, so more/smaller slices deliver
            # fewer early bytes and delay the PE's K=8/8 power window.
            # sync carries k=0, scalar k=1 - two queues ramp in parallel.
            nc.sync.dma_start(w1pk[:], w1pk_h.ap())
            for s0, s1 in ((0, 1024), (1024, 2048), (2048, 4096)):
                nc.sync.dma_start(dm[0][:, s0:s1],
                                  dataM_h.ap()[0:128, s0:s1])
            for s0, s1 in ((0, 1024), (1024, 2048), (2048, 4096)):
                nc.scalar.dma_start(dm[1][:, s0:s1],
                                    dataM_h.ap()[128:256, s0:s1])
            nc.gpsimd.dma_start(bpk[:], bpk_h.ap())
            nc.gpsimd.dma_start(w28sb[:, 0:2, :], w28_h.ap())
            nc.gpsimd.dma_start(w2pk[:], w2pk_h.ap())
            for s0, s1 in ((0, 2048), (2048, 4096)):
                for k in range(2):
                    nc.gpsimd.dma_start(
                        ds[k][:, s0:s1],
                        dataS_h.ap()[k * 128:(k + 1) * 128, s0:s1])
            for k in range(2):
                nc.gpsimd.dma_start(
                    dm[k][:, 4096:LC],
                    dataM_h.ap()[k * 128:(k + 1) * 128, 4096:LC])
            for k in range(2):
                nc.gpsimd.dma_start(
                    ds[k][:, 4096:LC],
                    dataS_h.ap()[k * 128:(k + 1) * 128, 4096:LC])

            # --- PE warmup: keep the PE busy from queue start so the DVFS
            # ramp to 2.4 GHz happens on scratch work ---
            for wi in range(24):
                pw = php.tile([128, CH], F32, tag="ph", name=f"warm{wi}")
                nc.tensor.matmul(
                    pw[:, 0:128], wscr[:], wscr[:], start=True, stop=True,
                )

            # --- main loop ---
            hbf = {}
            h8 = {}

            def fc1_block(g, ht):
                c0, c1 = GROUPS[g]
                # stationary w1 tile loaded once per (dt, ht); ph chunks of
                # 1024 cols so gelu runs 2-bank-wide
                ph = {}
                for dt in range(2):
                    for c in range(c0, c1):
                        if dt == 0:
                            ph[c] = php.tile([128, CH], F32, tag="ph",
                                             name=f"ph_{c}_{ht}")
                        for jj in range(2):
                            sl = slice(c * CH + jj * TILE,
                                       c * CH + (jj + 1) * TILE)
                            nc.tensor.matmul(
                                ph[c][:, jj * TILE:(jj + 1) * TILE],
                                w1s(dt, ht), dm[dt][:, sl],
                                start=(dt == 0), stop=(dt == 1),
                            )
                for c in range(c0, c1):
                    if ht < 2:
                        hb = hbfp.tile([128, CH], BF16, tag="hbf",
                                       name=f"hbf_{c}_{ht}")
                        dst = hb[:]
                        hbf[(c, ht)] = hb
                    else:
                        # h rows 256:512 -> fp8 planes for DoubleRow fc2
                        if ht == 2:
                            h8[c] = h8p.tile([128, 2, CH], F8, tag="h8",
                                             name=f"h8_{c}")
                        dst = h8[c][:, ht - 2, :]
                    nc.scalar.activation(
                        dst, ph[c][:],
                        mybir.ActivationFunctionType.Gelu,
                        bias=bpk[:, ht:ht + 1],
                    )

            # tail-out queues: the final output descriptors only reach
            # ~52 GB/s each (no queue depth), so the tail drains per-512
            # across several queues in parallel.  scalar only takes issues
            # for the very last group (its FIFO has no gelus left by then).
            TAIL_ENG = {4: (nc.sync, nc.gpsimd, nc.sync, nc.gpsimd),
                        5: (nc.scalar, nc.sync, nc.gpsimd, nc.scalar)}

            def fc2_piece(g, k, c):
                # one 1024-chunk of fc2 output half k: 6 matmuls + STT
                dsl = slice(k * 128, (k + 1) * 128)
                py_c = pyp.tile([128, CH], F32, tag="py", name=f"py_{c}_{k}")

                def mms(jj):
                    for ht in range(2):
                        nc.tensor.matmul(
                            py_c[:, jj * TILE:(jj + 1) * TILE],
                            w2s(ht, k),
                            hbf[(c, ht)][:, jj * TILE:(jj + 1) * TILE],
                            start=(ht == 0), stop=False,
                        )
                    nc.tensor.matmul(
                        py_c[:, jj * TILE:(jj + 1) * TILE],
                        w28sb[:, :, dsl],
                        h8[c][:, 0:2, jj * TILE:(jj + 1) * TILE],
                        start=False, stop=True,
                        perf_mode=mybir.MatmulPerfMode.DoubleRow,
                    )

                def stt(sl0, sl1):
                    chs = slice(c * CH + sl0, c * CH + sl1)
                    # acc = (y + b2) + rolled residual, bf16 out
                    nc.vector.scalar_tensor_tensor(
                        acc[k][:, chs], py_c[:, sl0:sl1],
                        bpk[:, 4 + k:5 + k], ds[k][:, chs],
                        mybir.AluOpType.add, mybir.AluOpType.add,
                    )
                    return chs

                if g >= 4:
                    # trailing groups: jj-major so each 512-half finishes
                    # (MMs -> STT -> out) independently; PE writes bank B
                    # while vector drains bank A
                    for jj in range(2):
                        mms(jj)
                        chs = stt(jj * TILE, (jj + 1) * TILE)
                        TAIL_ENG[g][2 * k + jj].dma_start(
                            outT_h.ap()[k * 128:(k + 1) * 128, chs],
                            acc[k][:, chs],
                        )
                    return
                for jj in range(2):
                    mms(jj)
                stt(0, CH)

            def out_block(g, eng):
                c0, c1 = GROUPS[g]
                bsl = slice(c0 * CH, c1 * CH)
                for k in range(2):
                    eng.dma_start(
                        outT_h.ap()[k * 128:(k + 1) * 128, bsl],
                        acc[k][:, bsl],
                    )

            NG = len(GROUPS)
            for g in range(NG + 1):
                # fc2 pieces of group g-1, interleaved one per fc1 block of
                # group g so the PE always has matmul work while gelu frees
                # the next ph buffer
                pieces = []
                if g > 0:
                    pc0, pc1 = GROUPS[g - 1]
                    pieces = [(k, c) for k in range(2) for c in range(pc0, pc1)]
                for ht in range(4):
                    if g < NG:
                        fc1_block(g, ht)
                    # big groups have 4 pieces (one per slot); small groups
                    # have 2 (emit on slots 1 and 3)
                    np_ = len(pieces)
                    if np_ == 4:
                        fc2_piece(g - 1, *pieces[ht])
                    elif np_ == 2 and ht in (1, 3):
                        fc2_piece(g - 1, *pieces[ht // 2])
                if g > 0 and g - 1 < 4:
                    # bulk output for the non-tail groups: sync is idle
                    # once its input slices are out (~18us); gpsimd's queue
                    # drains its input bulk by the time g2/g3 finish
                    out_block(g - 1, nc.sync if g - 1 < 2 else nc.gpsimd)

    nc.compile()
    return nc


_NC = None


def _get_nc():
    global _NC
    if _NC is None:
        _NC = _build()
    return _NC


def make_in_maps(data, w1, b1, w2, b2):
    data = np.asarray(data, dtype=np.float32)
    w1f = np.asarray(w1, dtype=np.float32)
    w2f = np.asarray(w2, dtype=np.float32)
    # w1 packed ht-major (see _build)
    w1pk = np.ascontiguousarray(np.concatenate(
        [w1f[dt * 128:(dt + 1) * 128, ht * 128:(ht + 1) * 128]
         for ht in range(4) for dt in range(2)], axis=1,
    )).astype(ml_dtypes.bfloat16)
    w2pk = np.ascontiguousarray(np.concatenate(
        [w2f[0:128, :], w2f[128:256, :]], axis=1,
    )).astype(ml_dtypes.bfloat16)
    # DoubleRow pack: [ki, ko, m] = w2[256 + 128*ko + ki, m]
    w28 = np.ascontiguousarray(
        w2f[H // 2:].reshape(2, 128, D).transpose(1, 0, 2)
    ).astype(ml_dtypes.float8_e4m3)
    # packed biases: cols 0:4 = b1 (col ht = rows ht*128..), cols 4:6 = b2
    bpk = np.ascontiguousarray(np.concatenate(
        [np.asarray(b1, dtype=np.float32).reshape(4, 128).T,
         np.asarray(b2, dtype=np.float32).reshape(2, 128).T], axis=1,
    ))

    in_maps = []
    for bb in range(B):
        # residual pre-rolled by +s_t per track:
        # rolled[l, c] = data[(l - s_t) mod L, c]
        rolled = np.empty((L, D), dtype=np.float32)
        for t in range(NT):
            cs = slice(t * TS, (t + 1) * TS)
            rolled[:, cs] = np.roll(data[bb, :, cs], SEFF[t], axis=0)
        for j in range(2):
            sl = slice(j * LC, (j + 1) * LC)
            dataM = np.ascontiguousarray(
                data[bb, sl, :].T.astype(ml_dtypes.bfloat16)
            )
            dataS = np.ascontiguousarray(
                rolled[sl, :].T.astype(ml_dtypes.bfloat16)
            )
            in_maps.append({
                "dataM": dataM, "dataS": dataS,
                "w1pk": w1pk, "w2pk": w2pk, "w28": w28, "bpk": bpk,
            })
    return in_maps


def kernel(data, w1, b1, w2, b2):
    nc = _get_nc()
    in_maps = make_in_maps(data, w1, b1, w2, b2)
    res = bass_utils.run_bass_kernel_spmd(
        nc, in_maps, core_ids=list(range(N_CORES))
    )
    out = np.empty((B, L, D), dtype=np.float32)
    for bb in range(B):
        # full[c, g] = out[(g - s_t(c)) mod L, c]; undo per-track rotation
        full = np.concatenate(
            [np.asarray(res.results[2 * bb + j]["outT"], dtype=np.float32)
             for j in range(2)], axis=1,
        )
        for t in range(NT):
            seg = full[t * TS:(t + 1) * TS, :]
            out[bb, :, t * TS:(t + 1) * TS] = np.roll(seg, -SEFF[t], axis=1).T
    return out


# revision 19
# speedup vs baseline: 1.0545x; 1.0080x over previous
"""ChordMixerBlock Trainium2 kernel.

Math (per batch b):
    h   = gelu(data @ w1 + b1)            # exact gelu
    y   = h @ w2 + b2
    out[l, :] = rotate_chord(y)[l, :] + data[l, :]
where rotate_chord rolls track t (channels [16t, 16t+16)) forward by
s_t = 2^(t-1) positions along L (track 0: no shift; track 15: 2^14 == L
-> no shift).

Sharding: 8 cores = (batch b, L-half j); each core computes y for its own
8192-token chunk in transposed layout [256 d, 8192 l] so the contraction
dim D lands on SBUF partitions (host pre-transposes inputs and transposes
the output back).

Roll handling is entirely layout-based -- no cross-core traffic:
  * acc[c, p] = y[c, p] + b2[c] + dataS[c, p], where dataS is the residual
    pre-rolled by +s_t per track on the HOST (pure sharding-layout prep).
    acc[c, p] is then exactly out[global (c0 + p - s_t) mod L, c] -- a
    complete output value, merely stored at a per-track rotated column.
  * Each core dumps acc verbatim; the HOST undoes the per-track column
    rotation while unsharding (np.roll per 16-channel track), so no
    collective and no boundary exchange is needed on device.

Device program per core (bf16 data path, fp32 accumulate in PSUM; the
upper half of the fc2 contraction runs as one fp8e4 DoubleRow matmul per
512-tile, keeping rel err ~1.5e-2 vs the 2e-2 budget):
  * All elementwise work runs on 1024-col (2-PSUM-bank) tiles: gelu on
    the scalar engine and the b2+residual STT on vector amortize their
    ~250ns per-instruction PSUM-access/seq overhead over twice the
    columns (scalar drops ~44us -> ~34us busy, below the PE's ~48us).
  * A dummy 1-col gelu right at program start pulls the ~1.3us
    ACT_TABLE_LOAD off the critical path (it otherwise serializes with
    the first real gelu).
  * w1 is packed ht-major so the first fc1 matmul only needs the first
    256 cols (64 KiB) of the weight stream plus one 512-col slice of dm.
  * Input DMA is spread over all three queues: sync (HWDGE) carries w1
    and dm cols 0:4096 front-loaded in fine slices, scalar (HWDGE)
    carries dm cols 4096:8192 in two big descriptors issued before the
    first gelu, gpsimd (SWDGE, ~2us fixed cost per op -> few big
    descriptors) carries bias/w2/fp8 packs then the rolled residual with
    its first 2048 cols leading.  This keeps the PE fed from ~1.5us
    after queue start with no mid-kernel stream stalls.
  * 16 dependency-free warmup matmuls on a zeroed scratch tile keep the
    PE busy from queue start so the DVFS ramp (0.65/1.2 -> 2.4 GHz after
    ~3us of continuous activity) burns on scratch, not real work.
  * Main loop: groups of l-chunks ([2048, 2048, 2048, 1024, 1024] cols),
    software-pipelined one group back (fc2(g-1) between fc1(g) blocks)
    so the PE never waits on the scalar engine's gelu chain; the two
    trailing 1024-col groups shorten the end-of-kernel gelu+STT tail.
  * gelu+bias on the scalar engine: h rows 0:256 -> bf16, rows 256:512
    -> fp8 planes of [128, 2, 1024] tiles consumed by DoubleRow matmuls.
  * Finished 2048-col output blocks stream out mid-kernel on the sync /
    gpsimd queues (idle once inputs are in); the last two groups finish
    STT+DMA per 1024-tile across all three queues to shorten the tail.
    Host upcasts bf16 -> fp32.
"""

import sys

sys.path.insert(0, "/opt/trn_rl_repo")

import numpy as np
import ml_dtypes

import concourse.bass as bass
import concourse.bacc as bacc
import concourse.tile as tile
import concourse.mybir as mybir
from concourse import bass_utils

B, L, D, H = 4, 16384, 256, 512
N_CORES = 8
LC = L // 2                      # per-core chunk length
NT, TS = 16, 16                  # tracks, track size
SHIFTS = [0] + [2 ** i for i in range(NT - 1)]
SEFF = [s % L for s in SHIFTS]   # track 15 -> 0
TILE = 512                       # max matmul output width
CH = 1024                        # elementwise chunk width (2 PSUM banks)
NCH = LC // CH                   # 8
# groups of 1024-col chunks; small leading groups let the PE start on
# less input data, small trailing groups shorten the end-of-kernel tail
GROUPS = [(0, 1), (1, 2), (2, 4), (4, 6), (6, 7), (7, 8)]

F32 = mybir.dt.float32
BF16 = mybir.dt.bfloat16
F8 = mybir.dt.float8e4


def _build():
    nc = bacc.Bacc(
        "TRN2", target_bir_lowering=False, debug=False,
        num_devices=N_CORES,
    )

    dataM_h = nc.dram_tensor("dataM", [D, LC], BF16, kind="ExternalInput")
    dataS_h = nc.dram_tensor("dataS", [D, LC], BF16, kind="ExternalInput")
    # w1 packed ht-major: cols [ht*256 + dt*128 :+128] = w1 rows
    # [dt*128:+128], cols [ht*128:+128] -- the first 256 cols are all the
    # first fc1 block needs, so it can start on ~64 KiB of weight stream.
    w1pk_h = nc.dram_tensor("w1pk", [128, 1024], BF16, kind="ExternalInput")
    # w2 cols [ht*256 + k*128 :+128] = w2 rows [ht*128:+128] cols [k*128:+128]
    w2pk_h = nc.dram_tensor("w2pk", [128, 512], BF16, kind="ExternalInput")
    # fc2 rows 256:512 as fp8 DoubleRow pack [ki, ko, m] = w2[256+128*ko+ki, m]
    w28_h = nc.dram_tensor("w28", [128, 2, D], F8, kind="ExternalInput")
    # cols 0:4 = b1 (col ht), cols 4:6 = b2 (col k)
    bpk_h = nc.dram_tensor("bpk", [128, 6], F32, kind="ExternalInput")
    outT_h = nc.dram_tensor("outT", [D, LC], BF16, kind="ExternalOutput")

    with tile.TileContext(nc) as tc:
        with (
            tc.tile_pool(name="const", bufs=1) as cpool,
            tc.tile_pool(name="big", bufs=1) as big,
            tc.tile_pool(name="hbf", bufs=8) as hbfp,
            tc.tile_pool(name="h8", bufs=4) as h8p,
            tc.tile_pool(name="ph", bufs=2, space="PSUM") as php,
            tc.tile_pool(name="py", bufs=2, space="PSUM") as pyp,
        ):
            # --- scratch + early gelu table load ---
            wscr = cpool.tile([128, 128], BF16, tag="wscr")
            ascr = cpool.tile([128, 2], F32, tag="ascr")
            nc.gpsimd.memset(ascr[:], 0)
            nc.gpsimd.memset(wscr[:], 0)
            # dummy 1-col gelu: forces ACT_TABLE_LOAD now, in parallel with
            # the input DMA, instead of before the first real gelu
            nc.scalar.activation(
                ascr[:, 1:2], ascr[:, 0:1],
                mybir.ActivationFunctionType.Gelu, bias=0.0,
            )

            w1pk = cpool.tile([128, 1024], BF16, tag="w1pk")
            w2pk = cpool.tile([128, 512], BF16, tag="w2pk")
            w28sb = cpool.tile([128, 2, D], F8, tag="w28")
            bpk = cpool.tile([128, 6], F32, tag="bpk")

            def w1s(dt, ht):
                o = ht * 256 + dt * 128
                return w1pk[:, o:o + 128]

            def w2s(ht, k):
                o = ht * 256 + k * 128
                return w2pk[:, o:o + 128]

            # --- persistent chunk buffers ---
            dm = [big.tile([128, LC], BF16, tag=f"dm{k}", name=f"dm{k}")
                  for k in range(2)]
            ds = [big.tile([128, LC], BF16, tag=f"ds{k}", name=f"ds{k}")
                  for k in range(2)]
            acc = [big.tile([128, LC], BF16, tag=f"acc{k}", name=f"acc{k}")
                   for k in range(2)]

            # --- input DMA ---
            # The fabric delivers little in the first ~8us (descriptor
            # pipeline depth builds slowly) and SDMA round-robin starves
            # small-run queues when another queue moves big runs.  So: the
            # PE-critical stream (w1 + dm cols 0:4096) is split across BOTH
            # HWDGE queues (sync: k=0, scalar: k=1) so two queues build
            # depth in parallel with nothing big competing; every bulk
            # half that is only needed from ~t+30us rides gpsimd (SWDGE)
            # behind the small packs, ordered by deadline.
            # Early HWDGE descriptors must be FEW: per-descriptor overhead
            # dominates the first ~10us, so more/smaller slices deliver
            # fewer early bytes and delay the PE's K=8/8 power window.
            # sync carries k=0, scalar k=1 - two queues ramp in parallel.
            nc.sync.dma_start(w1pk[:], w1pk_h.ap())
            for s0, s1 in ((0, 1024), (1024, 2048), (2048, 4096)):
                nc.sync.dma_start(dm[0][:, s0:s1],
                                  dataM_h.ap()[0:128, s0:s1])
            for s0, s1 in ((0, 1024), (1024, 2048), (2048, 4096)):
                nc.scalar.dma_start(dm[1][:, s0:s1],
                                    dataM_h.ap()[128:256, s0:s1])
            nc.gpsimd.dma_start(bpk[:], bpk_h.ap())
            nc.gpsimd.dma_start(w28sb[:, 0:2, :], w28_h.ap())
            nc.gpsimd.dma_start(w2pk[:], w2pk_h.ap())
            for s0, s1 in ((0, 2048), (2048, 4096)):
                for k in range(2):
                    nc.gpsimd.dma_start(
                        ds[k][:, s0:s1],
                        dataS_h.ap()[k * 128:(k + 1) * 128, s0:s1])
            for k in range(2):
                nc.gpsimd.dma_start(
                    dm[k][:, 4096:LC],
                    dataM_h.ap()[k * 128:(k + 1) * 128, 4096:LC])
            for k in range(2):
                nc.gpsimd.dma_start(
                    ds[k][:, 4096:LC],
                    dataS_h.ap()[k * 128:(k + 1) * 128, 4096:LC])

            # --- PE warmup: keep the PE busy from queue start so the DVFS
            # ramp to 2.4 GHz happens on scratch work ---
            for wi in range(24):
                pw = php.tile([128, CH], F32, tag="ph", name=f"warm{wi}")
                nc.tensor.matmul(
                    pw[:, 0:128], wscr[:], wscr[:], start=True, stop=True,
                )

            # --- main loop ---
            hbf = {}
            h8 = {}

            def fc1_block(g, ht):
                c0, c1 = GROUPS[g]
                # stationary w1 tile loaded once per (dt, ht); ph chunks of
                # 1024 cols so gelu runs 2-bank-wide
                ph = {}
                for dt in range(2):
                    for c in range(c0, c1):
                        if dt == 0:
                            ph[c] = php.tile([128, CH], F32, tag="ph",
                                             name=f"ph_{c}_{ht}")
                        for jj in range(2):
                            sl = slice(c * CH + jj * TILE,
                                       c * CH + (jj + 1) * TILE)
                            nc.tensor.matmul(
                                ph[c][:, jj * TILE:(jj + 1) * TILE],
                                w1s(dt, ht), dm[dt][:, sl],
                                start=(dt == 0), stop=(dt == 1),
                            )
                for c in range(c0, c1):
                    if ht < 2:
                        hb = hbfp.tile([128, CH], BF16, tag="hbf",
                                       name=f"hbf_{c}_{ht}")
                        dst = hb[:]
                        hbf[(c, ht)] = hb
                    else:
                        # h rows 256:512 -> fp8 planes for DoubleRow fc2
                        if ht == 2:
                            h8[c] = h8p.tile([128, 2, CH], F8, tag="h8",
                                             name=f"h8_{c}")
                        dst = h8[c][:, ht - 2, :]
                    nc.scalar.activation(
                        dst, ph[c][:],
                        mybir.ActivationFunctionType.Gelu,
                        bias=bpk[:, ht:ht + 1],
                    )

            # tail-out queues: the final output descriptors only reach
            # ~52 GB/s each (no queue depth), so the tail drains per-512
            # across several queues in parallel.  scalar only takes issues
            # for the very last group (its FIFO has no gelus left by then).
            TAIL_ENG = {4: (nc.sync, nc.gpsimd, nc.sync, nc.gpsimd),
                        5: (nc.scalar, nc.sync, nc.gpsimd, nc.sync)}

            def fc2_piece(g, k, c):
                # one 1024-chunk of fc2 output half k: 6 matmuls + STT
                dsl = slice(k * 128, (k + 1) * 128)
                py_c = pyp.tile([128, CH], F32, tag="py", name=f"py_{c}_{k}")

                def mms(jj):
                    for ht in range(2):
                        nc.tensor.matmul(
                            py_c[:, jj * TILE:(jj + 1) * TILE],
                            w2s(ht, k),
                            hbf[(c, ht)][:, jj * TILE:(jj + 1) * TILE],
                            start=(ht == 0), stop=False,
                        )
                    nc.tensor.matmul(
                        py_c[:, jj * TILE:(jj + 1) * TILE],
                        w28sb[:, :, dsl],
                        h8[c][:, 0:2, jj * TILE:(jj + 1) * TILE],
                        start=False, stop=True,
                        perf_mode=mybir.MatmulPerfMode.DoubleRow,
                    )

                def stt(sl0, sl1):
                    chs = slice(c * CH + sl0, c * CH + sl1)
                    # acc = (y + b2) + rolled residual, bf16 out
                    nc.vector.scalar_tensor_tensor(
                        acc[k][:, chs], py_c[:, sl0:sl1],
                        bpk[:, 4 + k:5 + k], ds[k][:, chs],
                        mybir.AluOpType.add, mybir.AluOpType.add,
                    )
                    return chs

                if g >= 4:
                    # trailing groups: jj-major so each 512-half finishes
                    # (MMs -> STT -> out) independently; PE writes bank B
                    # while vector drains bank A.  The very last chunk
                    # splits each out into 256-col halves on two queues --
                    # a single tail descriptor only reaches ~52 GB/s.
                    last = g == NG - 1
                    for jj in range(2):
                        mms(jj)
                        chs = stt(jj * TILE, (jj + 1) * TILE)
                        eng = TAIL_ENG[g][2 * k + jj]
                        if last:
                            eng2 = TAIL_ENG[g][(2 * k + jj + 1) % 4]
                            mid = chs.start + 256
                            eng.dma_start(
                                outT_h.ap()[k * 128:(k + 1) * 128,
                                            chs.start:mid],
                                acc[k][:, chs.start:mid],
                            )
                            eng2.dma_start(
                                outT_h.ap()[k * 128:(k + 1) * 128,
                                            mid:chs.stop],
                                acc[k][:, mid:chs.stop],
                            )
                        else:
                            eng.dma_start(
                                outT_h.ap()[k * 128:(k + 1) * 128, chs],
                                acc[k][:, chs],
                            )
                    return
                for jj in range(2):
                    mms(jj)
                stt(0, CH)

            def out_block(g, eng):
                c0, c1 = GROUPS[g]
                bsl = slice(c0 * CH, c1 * CH)
                for k in range(2):
                    eng.dma_start(
                        outT_h.ap()[k * 128:(k + 1) * 128, bsl],
                        acc[k][:, bsl],
                    )

            NG = len(GROUPS)
            for g in range(NG + 1):
                # fc2 pieces of group g-1, interleaved one per fc1 block of
                # group g so the PE always has matmul work while gelu frees
                # the next ph buffer
                pieces = []
                if g > 0:
                    pc0, pc1 = GROUPS[g - 1]
                    pieces = [(k, c) for k in range(2) for c in range(pc0, pc1)]
                for ht in range(4):
                    # 2-piece iterations put the piece BEFORE fc1 blocks 1/2
                    # so it covers the PE's wait for gelu to free the
                    # previous ph buffer
                    np_ = len(pieces)
                    if np_ == 2 and ht in (1, 2):
                        fc2_piece(g - 1, *pieces[ht - 1])
                    if g < NG:
                        fc1_block(g, ht)
                    if np_ == 4:
                        fc2_piece(g - 1, *pieces[ht])
                if g > 0 and g - 1 < 4:
                    # bulk output for the non-tail groups: sync is idle
                    # once its input slices are out (~18us); gpsimd's queue
                    # drains its input bulk by the time g2/g3 finish
                    out_block(g - 1, nc.sync if g - 1 < 2 else nc.gpsimd)

    nc.compile()
    return nc


_NC = None


def _get_nc():
    global _NC
    if _NC is None:
        _NC = _build()
    return _NC


def make_in_maps(data, w1, b1, w2, b2):
    data = np.asarray(data, dtype=np.float32)
    w1f = np.asarray(w1, dtype=np.float32)
    w2f = np.asarray(w2, dtype=np.float32)
    # w1 packed ht-major (see _build)
    w1pk = np.ascontiguousarray(np.concatenate(
        [w1f[dt * 128:(dt + 1) * 128, ht * 128:(ht + 1) * 128]
         for ht in range(4) for dt in range(2)], axis=1,
    )).astype(ml_dtypes.bfloat16)
    w2pk = np.ascontiguousarray(np.concatenate(
        [w2f[0:128, :], w2f[128:256, :]], axis=1,
    )).astype(ml_dtypes.bfloat16)
    # DoubleRow pack: [ki, ko, m] = w2[256 + 128*ko + ki, m]
    w28 = np.ascontiguousarray(
        w2f[H // 2:].reshape(2, 128, D).transpose(1, 0, 2)
    ).astype(ml_dtypes.float8_e4m3)
    # packed biases: cols 0:4 = b1 (col ht = rows ht*128..), cols 4:6 = b2
    bpk = np.ascontiguousarray(np.concatenate(
        [np.asarray(b1, dtype=np.float32).reshape(4, 128).T,
         np.asarray(b2, dtype=np.float32).reshape(2, 128).T], axis=1,
    ))

    in_maps = []
    for bb in range(B):
        # residual pre-rolled by +s_t per track:
        # rolled[l, c] = data[(l - s_t) mod L, c]
        rolled = np.empty((L, D), dtype=np.float32)
        for t in range(NT):
            cs = slice(t * TS, (t + 1) * TS)
            rolled[:, cs] = np.roll(data[bb, :, cs], SEFF[t], axis=0)
        for j in range(2):
            sl = slice(j * LC, (j + 1) * LC)
            dataM = np.ascontiguousarray(
                data[bb, sl, :].T.astype(ml_dtypes.bfloat16)
            )
            dataS = np.ascontiguousarray(
                rolled[sl, :].T.astype(ml_dtypes.bfloat16)
            )
            in_maps.append({
                "dataM": dataM, "dataS": dataS,
                "w1pk": w1pk, "w2pk": w2pk, "w28": w28, "bpk": bpk,
            })
    return in_maps


def kernel(data, w1, b1, w2, b2):
    nc = _get_nc()
    in_maps = make_in_maps(data, w1, b1, w2, b2)
    res = bass_utils.run_bass_kernel_spmd(
        nc, in_maps, core_ids=list(range(N_CORES))
    )
    out = np.empty((B, L, D), dtype=np.float32)
    for bb in range(B):
        # full[c, g] = out[(g - s_t(c)) mod L, c]; undo per-track rotation
        full = np.concatenate(
            [np.asarray(res.results[2 * bb + j]["outT"], dtype=np.float32)
             for j in range(2)], axis=1,
        )
        for t in range(NT):
            seg = full[t * TS:(t + 1) * TS, :]
            out[bb, :, t * TS:(t + 1) * TS] = np.roll(seg, -SEFF[t], axis=1).T
    return out
